# revision 1
# baseline (speedup 1.0000x reference)
"""GATv2-style masked attention kernel for Trainium2, 8-core data-parallel over batch.

Per core (one batch element, N=2048 nodes, F=256 features):
  h = x @ W                              (PE, fp32r)
  s_src = h @ a[:F], s_dst = h @ a[F:]   (PE, fused into the same matmuls)
  e[i,j] = leaky_relu(s_src[i] + s_dst[j], 0.2), masked by A
  alpha = softmax_j(e); y = alpha @ h

Softmax without row maxima: any per-i factor (and any global factor) cancels
in the normalization y = (P @ [h|1]) -> y[:, :F] / y[:, F], so we use
  P[j,i] = exp(leaky(u) - s_src_i - 54)
         = exp(max(-0.8*s_src_i, 0.8*s_dst_j) + 0.2*s_dst_j - 54)
with u = s_src_i + s_dst_j. The -54 recenters args near the typical row max
(3.4*sigma with sigma = ||W @ a_dst|| ~= 16 for this randn input spec) so the
fp16 score tiles keep precision where the big softmax weights live; bf16 P and
fp32 PSUM absorb the residual range with no under/overflow for any plausible
draw. The mask is applied multiplicatively after exp.

Scores are built transposed ([j, i]) so the P @ h contraction has j on
partitions. The i range is processed in two waves of 8 PSUM banks each, with
the mask resident in SBUF, so the P@h matmuls fully overlap score production.
The host supplies: x transposed, the mask transposed as bf16 {0,1}, W with the
attention vectors folded in ([W | W@a_src | W@a_dst]), and W@a_src replicated
across 128 columns (pure layout/weight transforms of the inputs).
"""

import numpy as np

B, N, F = 8, 2048, 256
PC = N // 128        # 16 j-chunks
KC = F // 128        # 2 contraction chunks for h
HALF = N // 2
_CACHE = {}


def _build():
    if "nc" in _CACHE:
        return _CACHE["nc"]

    from contextlib import ExitStack
    import concourse.bacc as bacc
    import concourse.tile as tile
    import concourse.mybir as mybir

    dt = mybir.dt
    AF = mybir.ActivationFunctionType
    ALU = mybir.AluOpType

    nc = bacc.Bacc("TRN2", target_bir_lowering=False, debug=False, num_devices=B)

    xT = nc.dram_tensor("xT", [F, N], dt.float32r, kind="ExternalInput").ap()
    Wsd = nc.dram_tensor("Wsd", [F, F + 2], dt.float32r, kind="ExternalInput").ap()
    Wrep = nc.dram_tensor("Wrep", [F, 128], dt.float32r, kind="ExternalInput").ap()
    maskT = nc.dram_tensor("maskT", [N, N], dt.bfloat16, kind="ExternalInput").ap()
    y = nc.dram_tensor("y", [N, F], dt.float32, kind="ExternalOutput").ap()


    with tile.TileContext(nc) as tc, ExitStack() as ctx:
        sb = ctx.enter_context(tc.tile_pool(name="sb", bufs=1))
        tpool = ctx.enter_context(tc.tile_pool(name="tp", bufs=6))
        p0pool = ctx.enter_context(tc.tile_pool(name="p0", bufs=4))
        phpool = ctx.enter_context(tc.tile_pool(name="ph", bufs=8))
        ypool = ctx.enter_context(tc.tile_pool(name="ysb", bufs=2))
        spool = ctx.enter_context(tc.tile_pool(name="small", bufs=4))
        ps = ctx.enter_context(tc.tile_pool(name="ps", bufs=8, space="PSUM"))

        # ---- persistent SBUF tensors ----
        xT_q = [
            sb.tile([128, KC, 512], dt.float32r, tag=f"xT{i}", name=f"xT{i}")
            for i in range(4)
        ]
        Wsd_t = sb.tile([128, KC, F + 2], dt.float32r, tag="Wsd")
        Wrep_t = sb.tile([128, KC, 128], dt.float32r, tag="Wrep")
        maskS = [
            sb.tile([128, HALF], dt.bfloat16, tag=f"maskS{j}", name=f"maskS{j}")
            for j in range(2 * PC)
        ]  # index w*PC + k -> strip k, i-half w
        hh = sb.tile([128, PC, F + 2], dt.float16, tag="hh")    # [h | 1] per chunk
        Sneg = [
            sb.tile([128, HALF], dt.float16, tag=f"Sneg{i}", name=f"Sneg{i}")
            for i in range(2)
        ]  # -0.8*s_src replicated, per i-half
        c2 = [
            sb.tile([128, 8], dt.float32, tag=f"c2_{i}", name=f"c2_{i}")
            for i in range(2)
        ]  # 0.8*s_dst, chunks 0-7 / 8-15
        bias2 = [
            sb.tile([128, 8], dt.float32, tag=f"bias2_{i}", name=f"bias2_{i}")
            for i in range(2)
        ]  # 0.2*s_dst - 54

        nc.vector.memset(hh[:, :, F : F + 1], 1.0)

        # ---- x loads pipelined with h-matmuls; s_src replication per segment ----
        def emit_seg_mm(seg):
            # replicated s_src segment: [128, 512]
            rp = ps.tile([128, 512], dt.float32, tag="bank", name=f"rep{seg}")
            for c in range(KC):
                nc.tensor.matmul(
                    rp[:],
                    Wrep_t[:, c, :],
                    xT_q[seg][:, c, :],
                    start=(c == 0),
                    stop=(c == KC - 1),
                )
            return rp

        def emit_seg_drain(seg, rp, on_act=False):
            half, off = divmod(seg * 512, HALF)
            dst = Sneg[half][:, off : off + 512]
            if on_act:
                nc.scalar.mul(dst, rp[:], -0.8)
            else:
                nc.vector.tensor_scalar_mul(dst, rp[:], -0.8)

        def emit_seg(seg):
            emit_seg_drain(seg, emit_seg_mm(seg))

        xTr = xT.rearrange("(c p) n -> p c n", p=128)

        def load_mask(w, j):
            nc.sync.dma_start(
                maskS[w * PC + j][:],
                maskT[j * 128 : (j + 1) * 128, w * HALF : (w + 1) * HALF],
            )

        # DMA order tuned so each consumer's data lands just ahead of its use:
        # x quarters feed the preamble matmuls, wave-1 masks interleave behind
        nc.sync.dma_start(xT_q[0][:], xTr[:, :, 0:512])
        nc.sync.dma_start(Wsd_t[:], Wsd.rearrange("(c p) m -> p c m", p=128))
        nc.sync.dma_start(Wrep_t[:], Wrep.rearrange("(c p) m -> p c m", p=128))
        nc.sync.dma_start(xT_q[1][:], xTr[:, :, 512:1024])
        load_mask(0, 0)
        load_mask(0, 1)
        nc.sync.dma_start(xT_q[2][:], xTr[:, :, 1024:1536])
        load_mask(0, 2)
        load_mask(0, 3)
        nc.sync.dma_start(xT_q[3][:], xTr[:, :, 1536:2048])
        for j in range(4, PC):
            load_mask(0, j)
        for j in range(PC):
            load_mask(1, j)
        def emit_h_mm(n_):
            q, off = divmod(n_ * 128, 512)
            hb = ps.tile([128, F + 2], dt.float32, tag="bank", name=f"hb{n_}")
            for c in range(KC):
                nc.tensor.matmul(
                    hb[:],
                    xT_q[q][:, c, off : off + 128],
                    Wsd_t[:, c, :],
                    start=(c == 0),
                    stop=(c == KC - 1),
                )
            return hb

        def emit_h_drain(n_, hb, on_act):
            if on_act:
                nc.scalar.copy(hh[:, n_, 0:F], hb[:, 0:F])
            else:
                nc.vector.tensor_copy(hh[:, n_, 0:F], hb[:, 0:F])
            g, col = divmod(n_, 8)
            nc.vector.tensor_scalar_mul(c2[g][:, col : col + 1], hb[:, F : F + 1], 0.8)
            nc.vector.tensor_scalar(
                bias2[g][:, col : col + 1], hb[:, F : F + 1], 0.2, -54.0,
                op0=ALU.mult, op1=ALU.add,
            )

        hb_late = {}
        for n_ in range(PC):
            hb = emit_h_mm(n_)
            if n_ < 8:
                emit_h_drain(n_, hb, on_act=True)
            else:
                hb_late[n_] = hb
            if n_ == 3:
                emit_seg(0)
            elif n_ == 7:
                emit_seg(1)
        rp_late = {seg: emit_seg_mm(seg) for seg in (2, 3)}

        # ---- normalize + store (staged; one output DMA per wave) ----
        def emit_norm(ysb, sl, bank, on_act):
            rec = spool.tile([128, 1], dt.float32, tag="rec")
            nc.vector.reciprocal(rec[:], bank[:, F : F + 1])
            if on_act:
                nc.scalar.activation(ysb[:, sl, :], bank[:, 0:F], AF.Copy, bias=0.0, scale=rec[:, 0:1])
            else:
                nc.vector.tensor_scalar_mul(ysb[:, sl, :], bank[:, 0:F], rec[:, 0:1])

        # ---- two waves over i-halves; strips over j-chunks.
        # Chunk 8-15 preamble drains are woven into wave-1's early strips so
        # the exp stream starts as soon as the first x quarter lands. Wave-2's
        # first score strips are emitted before wave-1's norms so ACT/DVE keep
        # streaming through the wave boundary while the norms wait on the
        # final wave-1 matmuls.
        def make_ts(w, k):
            g, col = divmod(k, 8)
            t = tpool.tile([128, HALF], dt.float16, tag="t", name=f"t{w}_{k}")
            nc.vector.tensor_scalar(
                t[:], Sneg[w][:], c2[g][:, col : col + 1], bias2[g][:, col : col + 1],
                op0=ALU.max, op1=ALU.add,
            )
            return t

        def make_scores(w, k, t):
            p0 = p0pool.tile([128, HALF], dt.bfloat16, tag="p0", name=f"p0_{w}_{k}")
            nc.scalar.activation(p0[:], t[:], AF.Exp, bias=0.0, scale=1.0)
            ph = phpool.tile([128, HALF], dt.bfloat16, tag="ph", name=f"ph{w}_{k}")
            nc.vector.tensor_mul(ph[:], p0[:], maskS[w * PC + k][:])
            return ph

        def emit_mms(banks, ph, k):
            for ic in range(8):
                nc.tensor.matmul(
                    banks[ic][:, 0 : F + 1],
                    ph[:, ic * 128 : (ic + 1) * 128],
                    hh[:, k, 0 : F + 1],
                    start=(k == 0),
                    stop=(k == PC - 1),
                )

        def emit_norms(w, ybanks, i0):
            for hlf in range(2):
                ysb = ypool.tile([128, 4, F], dt.float32, tag="ysb", name=f"ysb{w}_{hlf}")
                for ic in range(4):
                    g = hlf * 4 + ic
                    emit_norm(ysb, ic, ybanks[g], on_act=(g % 2 == 0))
                lo = i0 + hlf * 512
                nc.sync.dma_start(
                    y[lo : lo + 512, :].rearrange("(c p) f -> p c f", p=128), ysb[:]
                )

        # wave 1
        ybanks1 = [
            ps.tile([128, F + 2], dt.float32, tag="bank", name=f"yb0_{i}")
            for i in range(8)
        ]
        t_next = make_ts(0, 0)
        for k in range(PC):
            t = t_next
            if k + 1 < PC:
                pass
            p0 = p0pool.tile([128, HALF], dt.bfloat16, tag="p0", name=f"p0_0_{k}")
            nc.scalar.activation(p0[:], t[:], AF.Exp, bias=0.0, scale=1.0)
            if k + 1 < PC:
                t_next = make_ts(0, k + 1)
            ph = phpool.tile([128, HALF], dt.bfloat16, tag="ph", name=f"ph0_{k}")
            nc.vector.tensor_mul(ph[:], p0[:], maskS[k][:])
            if k in (0, 1):
                emit_seg_drain(k + 2, rp_late.pop(k + 2), on_act=True)
            if (k + 6) in hb_late:
                emit_h_drain(k + 6, hb_late.pop(k + 6), on_act=False)
            emit_mms(ybanks1, ph, k)

        # wave-2 head scores (pre-emitted across the boundary)
        ybanks2 = [
            ps.tile([128, F + 2], dt.float32, tag="bank", name=f"yb1_{i}")
            for i in range(8)
        ]
        HEAD = 4
        t2 = make_ts(1, 0)
        ph_head = []
        for k in range(HEAD):
            t = t2
            p0 = p0pool.tile([128, HALF], dt.bfloat16, tag="p0", name=f"p0_1_{k}")
            nc.scalar.activation(p0[:], t[:], AF.Exp, bias=0.0, scale=1.0)
            t2 = make_ts(1, k + 1)
            ph = phpool.tile([128, HALF], dt.bfloat16, tag="ph", name=f"ph1_{k}")
            nc.vector.tensor_mul(ph[:], p0[:], maskS[PC + k][:])
            ph_head.append(ph)

        emit_norms(0, ybanks1, 0)

        # wave 2 body
        for k in range(PC):
            if k < HEAD:
                ph = ph_head[k]
            else:
                t = t2
                p0 = p0pool.tile([128, HALF], dt.bfloat16, tag="p0", name=f"p0_1_{k}")
                nc.scalar.activation(p0[:], t[:], AF.Exp, bias=0.0, scale=1.0)
                if k + 1 < PC:
                    t2 = make_ts(1, k + 1)
                ph = phpool.tile([128, HALF], dt.bfloat16, tag="ph", name=f"ph1_{k}")
                nc.vector.tensor_mul(ph[:], p0[:], maskS[PC + k][:])
            emit_mms(ybanks2, ph, k)
        emit_norms(1, ybanks2, HALF)

    nc.compile()
    _CACHE["nc"] = nc
    return nc


def _prep_inputs(x, A, W, a):
    """Host-side layout transforms (per batch element)."""
    import ml_dtypes

    W32 = np.asarray(W, dtype=np.float32)
    a32 = np.asarray(a, dtype=np.float32)
    w_src = W32 @ a32[:F]
    w_dst = W32 @ a32[F:]
    Wsd = np.ascontiguousarray(
        np.concatenate([W32, w_dst[:, None], np.zeros((F, 1), np.float32)], axis=1),
        dtype=np.float32,
    )
    Wrep = np.ascontiguousarray(np.tile(w_src[:, None], (1, 128)), dtype=np.float32)
    in_maps = []
    for b in range(B):
        xTb = np.ascontiguousarray(np.asarray(x[b], dtype=np.float32).T)
        maskTb = np.ascontiguousarray((np.asarray(A[b]).T > 0).astype(ml_dtypes.bfloat16))
        in_maps.append({"xT": xTb, "Wsd": Wsd, "Wrep": Wrep, "maskT": maskTb})
    return in_maps


def kernel(x, A, W, a):
    from concourse.bass_utils import run_bass_kernel_spmd

    nc = _build()
    in_maps = _prep_inputs(x, A, W, a)
    res = run_bass_kernel_spmd(nc, in_maps, list(range(B)))
    out = np.stack([res.results[b]["y"] for b in range(B)]).astype(np.float32)
    return out



# revision 4
# speedup vs baseline: 1.2762x; 1.2762x over previous
"""GATv2-style masked attention kernel for Trainium2, 8-core data-parallel over batch.

Per core (one batch element, N=2048 nodes, F=256 features):
  h = x @ W                              (PE, fp16)
  s_src = h @ a[:F], s_dst = h @ a[F:]   (PE, fused into the same matmuls)
  e[i,j] = leaky_relu(s_src[i] + s_dst[j], 0.2), masked by A
  alpha = softmax_j(e); y = alpha @ h

Softmax without row maxima: any per-i factor cancels in the normalization
y = (P @ [h|1]) -> y[:, :F] / y[:, F].  Since exp is monotone,
  P[j,i] = exp(max(-0.8*s_src_i, 0.8*s_dst_j) + 0.2*s_dst_j - 54)
         = max(E_i, F_j) * G_j
with E_i = exp(-0.8*s_src_i - 27), F_j = exp(0.8*s_dst_j - 27),
G_j = exp(0.2*s_dst_j - 27): the N x N exp stream disappears entirely --
only 3 per-node exp vectors are needed.  Per score chunk the DVE computes
(E max F_j) * G_j as one 4x-mode tensor_scalar, then the {0,1} mask is
applied with one tensor_tensor (2x mode); a quarter of the mask multiplies
run on the otherwise-idle GPSIMD engine.  The -54 recentering (3.4*sigma
with sigma = ||W @ a_dst|| ~= 16 for this randn input spec) keeps the bf16
score tiles centered where the big softmax weights live.

Scores are built transposed ([j, i]) so the P @ h contraction has j on
partitions. The i range is processed in two waves of 8 PSUM banks each, with
the mask resident in SBUF.  The host supplies: x transposed in fp16, the mask
transposed as bf16 {0,1}, W with the attention vectors folded in
([W | W@a_dst | 0], fp16), and W@a_src replicated across 128 columns (pure
layout/precision transforms of the inputs).  y is stored bf16 and upcast on
the host.  A stream of tiny junk matmuls at t=0 keeps the PE busy through its
p-state ramp window so the real matmul stream runs at full clock.
"""

import numpy as np

B, N, F = 8, 2048, 256
PC = N // 128        # 16 j-chunks
KC = F // 128        # 2 contraction chunks for h
HALF = N // 2
NWARM = 52           # junk matmuls riding out the PE p-state ramp
_CACHE = {}


def _build():
    if "nc" in _CACHE:
        return _CACHE["nc"]

    from contextlib import ExitStack
    import concourse.bacc as bacc
    import concourse.tile as tile
    import concourse.mybir as mybir

    dt = mybir.dt
    AF = mybir.ActivationFunctionType
    ALU = mybir.AluOpType

    nc = bacc.Bacc("TRN2", target_bir_lowering=False, debug=False, num_devices=B)

    xT = nc.dram_tensor("xT", [F, N], dt.float16, kind="ExternalInput").ap()
    Wsd = nc.dram_tensor("Wsd", [F, F + 2], dt.float16, kind="ExternalInput").ap()
    Wrep = nc.dram_tensor("Wrep", [F, 128], dt.float16, kind="ExternalInput").ap()
    maskT = nc.dram_tensor("maskT", [N, N], dt.bfloat16, kind="ExternalInput").ap()
    y = nc.dram_tensor("y", [N, F], dt.bfloat16, kind="ExternalOutput").ap()

    with tile.TileContext(nc) as tc, ExitStack() as ctx:
        sb = ctx.enter_context(tc.tile_pool(name="sb", bufs=1))
        tpool = ctx.enter_context(tc.tile_pool(name="tp", bufs=6))
        phpool = ctx.enter_context(tc.tile_pool(name="ph", bufs=8))
        ypool = ctx.enter_context(tc.tile_pool(name="ysb", bufs=2))
        spool = ctx.enter_context(tc.tile_pool(name="small", bufs=4))
        ps = ctx.enter_context(tc.tile_pool(name="ps", bufs=8, space="PSUM"))

        # ---- persistent SBUF tensors ----
        xT_q = [
            sb.tile([128, KC, 512], dt.float16, tag=f"xT{i}", name=f"xT{i}")
            for i in range(4)
        ]
        Wsd_t = sb.tile([128, KC, F + 2], dt.float16, tag="Wsd")
        Wrep_t = sb.tile([128, KC, 128], dt.float16, tag="Wrep")
        maskS = [
            sb.tile([128, HALF], dt.bfloat16, tag=f"maskS{j}", name=f"maskS{j}")
            for j in range(2 * PC)
        ]  # index w*PC + k -> strip k, i-half w
        hh = sb.tile([128, PC, F + 2], dt.float16, tag="hh")    # [h | 1] per chunk
        E2 = [
            sb.tile([128, HALF], dt.bfloat16, tag=f"E2_{i}", name=f"E2_{i}")
            for i in range(2)
        ]  # exp(-0.8*s_src - 27) replicated, per i-half
        Fv = [
            sb.tile([128, 8], dt.float32, tag=f"Fv{i}", name=f"Fv{i}")
            for i in range(2)
        ]  # exp(0.8*s_dst - 27), chunks 0-7 / 8-15
        Gv = [
            sb.tile([128, 8], dt.float32, tag=f"Gv{i}", name=f"Gv{i}")
            for i in range(2)
        ]  # exp(0.2*s_dst - 27)
        cst = sb.tile([128, 4], dt.float32, tag="cst")  # b27, -0.8, 0.8, 0.2
        junk = sb.tile([128, 64], dt.float16, tag="junk")

        nc.gpsimd.memset(cst[:, 0:1], -27.0)
        nc.gpsimd.memset(cst[:, 1:2], -0.8)
        nc.gpsimd.memset(cst[:, 2:3], 0.8)
        nc.gpsimd.memset(cst[:, 3:4], 0.2)
        nc.gpsimd.memset(junk[:], 0.0)
        nc.vector.memset(hh[:, :, F : F + 1], 1.0)

        # ---- PE p-state warm-up: tiny junk matmuls occupy the ramp window ----
        warm = ps.tile([64, 64], dt.float32, tag="bank", name="warm")
        for _ in range(NWARM):
            nc.tensor.matmul(warm[:], junk[:, 0:64], junk[:, 0:64], start=True, stop=True)

        # ---- x loads pipelined with h-matmuls ----
        def emit_seg_mm(seg):
            # replicated s_src segment: [128, 512]
            rp = ps.tile([128, 512], dt.float32, tag="bank", name=f"rep{seg}")
            for c in range(KC):
                nc.tensor.matmul(
                    rp[:],
                    Wrep_t[:, c, :],
                    xT_q[seg][:, c, :],
                    start=(c == 0),
                    stop=(c == KC - 1),
                )
            return rp

        def emit_seg_drain(seg, rp):
            half, off = divmod(seg * 512, HALF)
            nc.scalar.activation(
                E2[half][:, off : off + 512], rp[:], AF.Exp,
                bias=cst[:, 0:1], scale=cst[:, 1:2],
            )

        def emit_seg(seg):
            emit_seg_drain(seg, emit_seg_mm(seg))

        xTr = xT.rearrange("(c p) n -> p c n", p=128)

        def load_mask(w, j):
            nc.sync.dma_start(
                maskS[w * PC + j][:],
                maskT[j * 128 : (j + 1) * 128, w * HALF : (w + 1) * HALF],
            )

        # DMA order tuned so each consumer's data lands just ahead of its use
        nc.sync.dma_start(xT_q[0][:], xTr[:, :, 0:512])
        nc.sync.dma_start(Wsd_t[:], Wsd.rearrange("(c p) m -> p c m", p=128))
        nc.sync.dma_start(Wrep_t[:], Wrep.rearrange("(c p) m -> p c m", p=128))
        nc.sync.dma_start(xT_q[1][:], xTr[:, :, 512:1024])
        load_mask(0, 0)
        load_mask(0, 1)
        nc.sync.dma_start(xT_q[2][:], xTr[:, :, 1024:1536])
        load_mask(0, 2)
        load_mask(0, 3)
        nc.sync.dma_start(xT_q[3][:], xTr[:, :, 1536:2048])
        for j in range(4, PC):
            load_mask(0, j)
        for j in range(PC):
            load_mask(1, j)

        def emit_h_mm(n_):
            q, off = divmod(n_ * 128, 512)
            hb = ps.tile([128, F + 2], dt.float32, tag="bank", name=f"hb{n_}")
            for c in range(KC):
                nc.tensor.matmul(
                    hb[:],
                    xT_q[q][:, c, off : off + 128],
                    Wsd_t[:, c, :],
                    start=(c == 0),
                    stop=(c == KC - 1),
                )
            return hb

        def emit_h_drain(n_, hb):
            nc.scalar.copy(hh[:, n_, 0:F], hb[:, 0:F])
            g, col = divmod(n_, 8)
            nc.scalar.activation(
                Fv[g][:, col : col + 1], hb[:, F : F + 1], AF.Exp,
                bias=cst[:, 0:1], scale=cst[:, 2:3],
            )
            nc.scalar.activation(
                Gv[g][:, col : col + 1], hb[:, F : F + 1], AF.Exp,
                bias=cst[:, 0:1], scale=cst[:, 3:4],
            )

        hb_late = {}
        for n_ in range(PC):
            hb = emit_h_mm(n_)
            if n_ < 8:
                emit_h_drain(n_, hb)
            else:
                hb_late[n_] = hb
            if n_ == 3:
                emit_seg(0)
            elif n_ == 7:
                emit_seg(1)
        rp_late = {seg: emit_seg_mm(seg) for seg in (2, 3)}

        # ---- normalize + store (staged; one output DMA per wave-half) ----
        def emit_norm(ysb, sl, bank, on_act):
            rec = spool.tile([128, 1], dt.float32, tag="rec")
            nc.vector.reciprocal(rec[:], bank[:, F : F + 1])
            if on_act:
                nc.scalar.activation(ysb[:, sl, :], bank[:, 0:F], AF.Copy, bias=0.0, scale=rec[:, 0:1])
            else:
                nc.vector.tensor_scalar_mul(ysb[:, sl, :], bank[:, 0:F], rec[:, 0:1])

        # ---- two waves over i-halves; strips over j-chunks ----
        def make_ts(w, k):
            g, col = divmod(k, 8)
            t = tpool.tile([128, HALF], dt.bfloat16, tag="t", name=f"t{w}_{k}")
            nc.vector.tensor_scalar(
                t[:], E2[w][:], Fv[g][:, col : col + 1], Gv[g][:, col : col + 1],
                op0=ALU.max, op1=ALU.mult,
            )
            return t

        def make_ph(w, k, t):
            ph = phpool.tile([128, HALF], dt.bfloat16, tag="ph", name=f"ph{w}_{k}")
            if k % 4 == 2:
                nc.gpsimd.tensor_tensor(ph[:], t[:], maskS[w * PC + k][:], op=ALU.mult)
            else:
                nc.vector.tensor_mul(ph[:], t[:], maskS[w * PC + k][:])
            return ph

        def emit_mms(banks, ph, k):
            for ic in range(8):
                nc.tensor.matmul(
                    banks[ic][:, 0 : F + 1],
                    ph[:, ic * 128 : (ic + 1) * 128],
                    hh[:, k, 0 : F + 1],
                    start=(k == 0),
                    stop=(k == PC - 1),
                )

        def emit_norms(w, ybanks, i0):
            for hlf in range(2):
                ysb = ypool.tile([128, 4, F], dt.bfloat16, tag="ysb", name=f"ysb{w}_{hlf}")
                for ic in range(4):
                    g = hlf * 4 + ic
                    emit_norm(ysb, ic, ybanks[g], on_act=(g % 2 == 0))
                lo = i0 + hlf * 512
                nc.sync.dma_start(
                    y[lo : lo + 512, :].rearrange("(c p) f -> p c f", p=128), ysb[:]
                )

        # wave 1
        ybanks1 = [
            ps.tile([128, F + 2], dt.float32, tag="bank", name=f"yb0_{i}")
            for i in range(8)
        ]
        t_next = make_ts(0, 0)
        for k in range(PC):
            t = t_next
            if k + 1 < PC:
                t_next = make_ts(0, k + 1)
            ph = make_ph(0, k, t)
            if k in (0, 1):
                emit_seg_drain(k + 2, rp_late.pop(k + 2))
            if (k + 6) in hb_late:
                emit_h_drain(k + 6, hb_late.pop(k + 6))
            emit_mms(ybanks1, ph, k)

        # wave-2 head scores (pre-emitted across the boundary)
        ybanks2 = [
            ps.tile([128, F + 2], dt.float32, tag="bank", name=f"yb1_{i}")
            for i in range(8)
        ]
        HEAD = 4
        t2 = make_ts(1, 0)
        ph_head = []
        for k in range(HEAD):
            t = t2
            t2 = make_ts(1, k + 1)
            ph_head.append(make_ph(1, k, t))

        emit_norms(0, ybanks1, 0)

        # wave 2 body
        for k in range(PC):
            if k < HEAD:
                ph = ph_head[k]
            else:
                t = t2
                if k + 1 < PC:
                    t2 = make_ts(1, k + 1)
                ph = make_ph(1, k, t)
            emit_mms(ybanks2, ph, k)
        emit_norms(1, ybanks2, HALF)

    nc.compile()
    _CACHE["nc"] = nc
    return nc


def _prep_inputs(x, A, W, a):
    """Host-side layout/precision transforms (per batch element)."""
    import ml_dtypes

    W32 = np.asarray(W, dtype=np.float32)
    a32 = np.asarray(a, dtype=np.float32)
    w_src = W32 @ a32[:F]
    w_dst = W32 @ a32[F:]
    Wsd = np.ascontiguousarray(
        np.concatenate([W32, w_dst[:, None], np.zeros((F, 1), np.float32)], axis=1)
    ).astype(np.float16)
    Wrep = np.ascontiguousarray(np.tile(w_src[:, None], (1, 128))).astype(np.float16)
    in_maps = []
    for b in range(B):
        xTb = np.ascontiguousarray(np.asarray(x[b], dtype=np.float32).T).astype(np.float16)
        maskTb = np.ascontiguousarray((np.asarray(A[b]).T > 0).astype(ml_dtypes.bfloat16))
        in_maps.append({"xT": xTb, "Wsd": Wsd, "Wrep": Wrep, "maskT": maskTb})
    return in_maps


def kernel(x, A, W, a):
    from concourse.bass_utils import run_bass_kernel_spmd

    nc = _build()
    in_maps = _prep_inputs(x, A, W, a)
    res = run_bass_kernel_spmd(nc, in_maps, list(range(B)))
    out = np.stack([np.asarray(res.results[b]["y"]).astype(np.float32) for b in range(B)])
    return out


# revision 52
# speedup vs baseline: 1.3964x; 1.0942x over previous
"""GATv2-style masked attention kernel for Trainium2, 8-core data-parallel over batch.

Per core (one batch element, N=2048 nodes, F=256 features):
  h = x @ W                              (PE, fp16)
  s_src = h @ a[:F], s_dst = h @ a[F:]   (PE, fused into the same matmuls)
  e[i,j] = leaky_relu(s_src[i] + s_dst[j], 0.2), masked by A
  alpha = softmax_j(e); y = alpha @ h

Softmax without row maxima: any per-i factor cancels in the normalization
y = (P @ [h|1]) -> y[:, :F] / y[:, F].  Since exp is monotone,
  P[j,i] = exp(max(-0.8*s_src_i, 0.8*s_dst_j) + 0.2*s_dst_j - 54)
         = max(E_i, F_j) * G_j
with E_i = exp(-0.8*s_src_i - 27), F_j = exp(0.8*s_dst_j - 27),
G_j = exp(0.2*s_dst_j - 27): the N x N exp stream disappears entirely --
only per-node exp vectors are needed.  Per score chunk the DVE computes
(E max F_j) * G_j as one 4x-mode tensor_scalar, then the {0,1} mask is
applied with one 2x tensor_tensor; some mask multiplies run on the
otherwise-idle GPSIMD engine.  The -54 recentering (3.4*sigma with
sigma = ||W @ a_dst|| ~= 16 for this randn input spec) keeps the bf16
score tiles centered where the big softmax weights live.

Scores are built transposed ([j, i]) so the P @ h contraction has j on
partitions.  The i range is processed in two waves of 8 PSUM banks each.
h is produced two i-chunks per PSUM bank ([W] only); s_dst accumulates in a
16-column bank via rank-1 matmuls against the w_dst column so the F/G exps
batch into four activations; h->SBUF drains are spread over ACT and DVE
(GPSIMD cannot touch PSUM) so the banks recycle quickly into the wave
accumulators.  The last few j-strips of each wave run bank-major so
normalization and the bf16 output stores pipeline behind the matmul tail
instead of serializing after it.  A stream of tiny junk matmuls at t=0
rides out the PE p-state ramp so the real matmul stream runs at full
clock.  The host supplies: x transposed in fp16, the mask transposed as
bf16 {0,1}, [W | W@a_dst | 0] in fp16, and W@a_src replicated across 128
columns (pure layout/precision transforms); y is stored bf16 and upcast
on the host.
"""

import numpy as np

B, N, F = 8, 2048, 256
PC = N // 128        # 16 j-chunks
CC = 2               # contraction chunks (f in two halves of 128)
HALF = N // 2
NWARM = 52
STAG = 4             # last STAG j-strips of each wave run bank-major
_CACHE = {}


def _build():
    if "nc" in _CACHE:
        return _CACHE["nc"]

    from contextlib import ExitStack
    import concourse.bacc as bacc
    import concourse.tile as tile
    import concourse.mybir as mybir

    dt = mybir.dt
    AF = mybir.ActivationFunctionType
    ALU = mybir.AluOpType

    nc = bacc.Bacc("TRN2", target_bir_lowering=False, debug=False, num_devices=B)

    xT = nc.dram_tensor("xT", [F, N], dt.float16, kind="ExternalInput").ap()
    Wsd = nc.dram_tensor("Wsd", [F, F + 2], dt.float16, kind="ExternalInput").ap()
    Wrep = nc.dram_tensor("Wrep", [F, 128], dt.float16, kind="ExternalInput").ap()
    maskT = nc.dram_tensor("maskT", [N, N], dt.bfloat16, kind="ExternalInput").ap()
    y = nc.dram_tensor("y", [N, F], dt.bfloat16, kind="ExternalOutput").ap()

    with tile.TileContext(nc) as tc, ExitStack() as ctx:
        sb = ctx.enter_context(tc.tile_pool(name="sb", bufs=1))
        tpool = ctx.enter_context(tc.tile_pool(name="tp", bufs=6))
        phpool = ctx.enter_context(tc.tile_pool(name="ph", bufs=8))
        ypool = ctx.enter_context(tc.tile_pool(name="ysb", bufs=8))
        spool = ctx.enter_context(tc.tile_pool(name="small", bufs=4))
        ps = ctx.enter_context(tc.tile_pool(name="ps", bufs=8, space="PSUM"))

        # ---- persistent SBUF tensors ----
        xh = [
            sb.tile([128, CC, 1024], dt.float16, tag=f"xh{i}", name=f"xh{i}")
            for i in range(2)
        ]  # x in two halves of 8 i-chunks each
        Wsd_t = sb.tile([128, CC, F + 2], dt.float16, tag="Wsd")
        Wrep_t = sb.tile([128, CC, 128], dt.float16, tag="Wrep")
        maskS = [
            sb.tile([128, HALF], dt.bfloat16, tag=f"maskS{j}", name=f"maskS{j}")
            for j in range(2 * PC)
        ]
        hh = sb.tile([128, PC, F + 2], dt.float16, tag="hh")    # [h | 1] per chunk
        E2 = [
            sb.tile([128, HALF], dt.bfloat16, tag=f"E2_{i}", name=f"E2_{i}")
            for i in range(2)
        ]  # exp(-0.8*s_src - 27) replicated, per i-half
        Fv = sb.tile([128, PC], dt.float32, tag="Fv")  # exp(0.8*s_dst - 27)
        Gv = sb.tile([128, PC], dt.float32, tag="Gv")  # exp(0.2*s_dst - 27)
        cst = sb.tile([128, 4], dt.float32, tag="cst")  # -27, -0.8, 0.8, 0.2
        junk = sb.tile([128, 64], dt.float16, tag="junk")

        nc.gpsimd.memset(junk[:], 0.0)
        nc.gpsimd.memset(cst[:, 0:1], -27.0)
        nc.gpsimd.memset(cst[:, 1:2], -0.8)
        nc.gpsimd.memset(cst[:, 2:3], 0.8)
        nc.gpsimd.memset(cst[:, 3:4], 0.2)
        nc.vector.memset(hh[:, :, F : F + 1], 1.0)
        # Pull the Exp activation table in before the DMA window closes.
        scr = spool.tile([128, 1], dt.float32, tag="rec", name="scr")
        nc.scalar.activation(scr[:], cst[:, 0:1], AF.Exp, bias=0.0, scale=1.0)

        # ---- PE p-state warm-up ----
        warm = ps.tile([64, 64], dt.float32, tag="bank", name="warm")
        for _ in range(NWARM):
            nc.tensor.matmul(warm[:], junk[:, 0:64], junk[:, 0:64], start=True, stop=True)

        # ---- DMAs ----
        xTr = xT.rearrange("(c p) n -> p c n", p=128)

        def load_mask(w, j):
            nc.sync.dma_start(
                maskS[w * PC + j][:],
                maskT[j * 128 : (j + 1) * 128, w * HALF : (w + 1) * HALF],
            )

        def load_xq(q):
            h, off = divmod(q * 512, 1024)
            nc.sync.dma_start(
                xh[h][:, :, off : off + 512], xTr[:, :, q * 512 : q * 512 + 512]
            )

        load_xq(0)
        nc.sync.dma_start(Wsd_t[:], Wsd.rearrange("(c p) m -> p c m", p=128))
        nc.sync.dma_start(Wrep_t[:], Wrep.rearrange("(c p) m -> p c m", p=128))
        load_xq(1)
        load_xq(2)
        load_mask(0, 0)
        load_xq(3)
        load_mask(0, 1)
        for j in range(2, PC):
            load_mask(0, j)
        for j in range(PC):
            load_mask(1, j)

        # ---- preamble: E / h / s_dst production ----
        def emit_seg_mm(seg):
            rp = ps.tile([128, 512], dt.float32, tag="bank", name=f"rep{seg}")
            for c in range(CC):
                nc.tensor.matmul(
                    rp[:],
                    Wrep_t[:, c, :],
                    xh[seg // 2][:, c, (seg % 2) * 512 : (seg % 2) * 512 + 512],
                    start=(c == 0),
                    stop=(c == CC - 1),
                )
            return rp

        def emit_seg_drain(seg, rp):
            half, off = divmod(seg * 512, HALF)
            nc.scalar.activation(
                E2[half][:, off : off + 512], rp[:], AF.Exp,
                bias=cst[:, 0:1], scale=cst[:, 1:2],
            )

        def emit_H_mm(nb):
            # paired h for chunks 2nb, 2nb+1 -> one [128, 512] bank
            hb = ps.tile([128, 512], dt.float32, tag="bank", name=f"H{nb}")
            for half in range(2):
                ch = 2 * nb + half
                q, off = divmod(ch * 128, 1024)
                for c in range(CC):
                    nc.tensor.matmul(
                        hb[:, half * 256 : half * 256 + 256],
                        xh[q][:, c, off : off + 128],
                        Wsd_t[:, c, 0:256],
                        start=(c == 0),
                        stop=(c == CC - 1),
                    )
            return hb

        def emit_sdb_mms(sdb, ch):
            q, off = divmod(ch * 128, 1024)
            for c in range(CC):
                nc.tensor.matmul(
                    sdb[:, ch : ch + 1],
                    xh[q][:, c, off : off + 128],
                    Wsd_t[:, c, F : F + 1],
                    start=(c == 0),
                    stop=(c == CC - 1),
                )

        def emit_hh_drain(nb, hb, eng):
            # GPSIMD cannot read PSUM -- ACT/DVE only here.
            dst = hh[:, 2 * nb : 2 * nb + 2, 0:F]
            if eng == "act":
                nc.scalar.copy(dst, hb[:])
            else:
                nc.vector.tensor_copy(dst, hb[:])

        def emit_fg(sdb, lo, hi):
            nc.scalar.activation(
                Fv[:, lo:hi], sdb[:, lo:hi], AF.Exp, bias=cst[:, 0:1], scale=cst[:, 2:3],
            )
            nc.scalar.activation(
                Gv[:, lo:hi], sdb[:, lo:hi], AF.Exp, bias=cst[:, 0:1], scale=cst[:, 3:4],
            )

        # x quarters 0,1: chunks 0-7, segs 0-1; sdb is long-lived
        H = {}
        H[0] = emit_H_mm(0)
        H[1] = emit_H_mm(1)
        rp0 = emit_seg_mm(0)
        emit_seg_drain(0, rp0)
        sdb = ps.tile([128, PC], dt.float32, tag="bank", name="sdb")
        for ch in range(0, 4):
            emit_sdb_mms(sdb, ch)
        emit_hh_drain(0, H[0], "dve")
        rp1 = emit_seg_mm(1)
        emit_seg_drain(1, rp1)
        H[2] = emit_H_mm(2)
        H[3] = emit_H_mm(3)
        for ch in range(4, 8):
            emit_sdb_mms(sdb, ch)
        emit_fg(sdb, 0, 8)
        emit_hh_drain(1, H[1], "dve")
        emit_hh_drain(2, H[2], "dve")

        # x quarter 2: seg 2, chunks 8-11
        rp2 = emit_seg_mm(2)
        emit_seg_drain(2, rp2)
        H[4] = emit_H_mm(4)
        H[5] = emit_H_mm(5)
        for ch in range(8, 12):
            emit_sdb_mms(sdb, ch)
        emit_hh_drain(3, H[3], "dve")

        # x quarter 3: seg 3, s_dst 12-15, chunks 12-15
        rp3 = emit_seg_mm(3)
        for ch in range(12, 16):
            emit_sdb_mms(sdb, ch)
        emit_fg(sdb, 8, 16)
        emit_seg_drain(3, rp3)
        H[6] = emit_H_mm(6)
        H[7] = emit_H_mm(7)
        emit_hh_drain(4, H[4], "act")
        emit_hh_drain(6, H[6], "act")
        emit_hh_drain(5, H[5], "act")
        # hh drain for bank 7 rides inside wave 1 on the DVE (below)

        # ---- score strips ----
        POOL_KS = ((3, 6, 9, 12), (6, 10))

        def make_ts(w, k):
            t = tpool.tile([128, HALF], dt.bfloat16, tag="t", name=f"t{w}_{k}")
            nc.vector.tensor_scalar(
                t[:], E2[w][:], Fv[:, k : k + 1], Gv[:, k : k + 1],
                op0=ALU.max, op1=ALU.mult,
            )
            return t

        def make_ph(w, k, t):
            ph = phpool.tile([128, HALF], dt.bfloat16, tag="ph", name=f"ph{w}_{k}")
            if k in POOL_KS[w]:
                # halves: the strip's first 4 bank-matmuls only need cols 0:512
                nc.gpsimd.tensor_tensor(ph[:, 0:512], t[:, 0:512], maskS[w * PC + k][:, 0:512], op=ALU.mult)
                nc.gpsimd.tensor_tensor(ph[:, 512:1024], t[:, 512:1024], maskS[w * PC + k][:, 512:1024], op=ALU.mult)
            else:
                nc.vector.tensor_mul(ph[:], t[:], maskS[w * PC + k][:])
            return ph

        def emit_mms(banks, ph, k, order=None):
            for ic in (order if order is not None else range(8)):
                nc.tensor.matmul(
                    banks[ic][:, 0 : F + 1],
                    ph[:, ic * 128 : (ic + 1) * 128],
                    hh[:, k, 0 : F + 1],
                    start=(k == 0),
                    stop=(k == PC - 1),
                )

        def emit_norm(ysb, sl, bank, eng):
            rec = spool.tile([128, 1], dt.float32, tag="rec")
            nc.vector.reciprocal(rec[:], bank[:, F : F + 1])
            if eng == "act":
                nc.scalar.activation(ysb[:, sl, :], bank[:, 0:F], AF.Copy, bias=0.0, scale=rec[:, 0:1])
            else:
                nc.vector.tensor_scalar_mul(ysb[:, sl, :], bank[:, 0:F], rec[:, 0:1])

        def emit_wave(w, ybanks, k0_order=None, dve_hooks=None):
            """j-strips 0..PC-1; last STAG strips bank-major with fused norms+stores."""
            t_next = make_ts(w, 0)
            for k in range(PC - STAG):
                t = t_next
                t_next = make_ts(w, k + 1)
                ph = make_ph(w, k, t)
                if dve_hooks and k in dve_hooks:
                    dve_hooks[k]()
                emit_mms(ybanks, ph, k, order=(k0_order if k == 0 else None))
            phs = {}
            for k in range(PC - STAG, PC):
                t = t_next
                if k + 1 < PC:
                    t_next = make_ts(w, k + 1)
                phs[k] = make_ph(w, k, t)
            groups = [(0, 2), (2, 2), (4, 2), (6, 2)] if w == 0 else [(0, 4), (4, 3), (7, 1)]
            ysbs = {g[0]: ypool.tile([128, g[1], F], dt.bfloat16, tag="ysb", name=f"ysb{w}_{g[0]}")
                    for g in groups}
            gof = {}
            for g0, gn in groups:
                for i in range(g0, g0 + gn):
                    gof[i] = (g0, gn)
            # wave 0's scale-copies all ride ACT so the DVE stream rolls
            # straight into wave 1's strips; the final wave alternates.
            engs = ("act",) if w == 0 else ("dve", "act")
            for ic in range(8):
                for k in range(PC - STAG, PC):
                    nc.tensor.matmul(
                        ybanks[ic][:, 0 : F + 1],
                        phs[k][:, ic * 128 : (ic + 1) * 128],
                        hh[:, k, 0 : F + 1],
                        start=False,
                        stop=(k == PC - 1),
                    )
                g0, gn = gof[ic]
                emit_norm(ysbs[g0], ic - g0, ybanks[ic], engs[ic % len(engs)])
                if ic == g0 + gn - 1:
                    lo = w * HALF + g0 * 128
                    nc.sync.dma_start(
                        y[lo : lo + gn * 128, :].rearrange("(c p) f -> p c f", p=128),
                        ysbs[g0][:],
                    )

        # wave 1 -- k0 banks ordered by PSUM-slot drain readiness; hh drains for
        # H5/H7 ride the DVE between early strips
        ybanks1 = [
            ps.tile([128, F + 2], dt.float32, tag="bank", name=f"yb0_{i}")
            for i in range(8)
        ]
        # ps slot children (alloc order warm,H0,H1,rep0,sdb,rep1,H2,H3,rep2,H4,H5,rep3,H6,H7):
        # yb0_0<-H2, yb0_1<-H3, yb0_2<-rep2, yb0_3<-H4, yb0_4<-H5, yb0_5<-rep3, yb0_6<-H6, yb0_7<-H7
        hooks = {
            3: lambda: emit_hh_drain(7, H[7], "dve"),
        }
        emit_wave(0, ybanks1, k0_order=[0, 2, 1, 3, 5, 6, 4, 7], dve_hooks=hooks)
        ybanks2 = [
            ps.tile([128, F + 2], dt.float32, tag="bank", name=f"yb1_{i}")
            for i in range(8)
        ]
        emit_wave(1, ybanks2)

    nc.compile()
    _CACHE["nc"] = nc
    return nc


def _prep_inputs(x, A, W, a):
    """Host-side layout/precision transforms (per batch element)."""
    import ml_dtypes

    W32 = np.asarray(W, dtype=np.float32)
    a32 = np.asarray(a, dtype=np.float32)
    w_src = W32 @ a32[:F]
    w_dst = W32 @ a32[F:]
    Wsd = np.ascontiguousarray(
        np.concatenate([W32, w_dst[:, None], np.zeros((F, 1), np.float32)], axis=1)
    ).astype(np.float16)
    Wrep = np.ascontiguousarray(np.tile(w_src[:, None], (1, 128))).astype(np.float16)
    in_maps = []
    for b in range(B):
        xTb = np.ascontiguousarray(np.asarray(x[b], dtype=np.float32).T).astype(np.float16)
        maskTb = np.ascontiguousarray((np.asarray(A[b]).T > 0).astype(ml_dtypes.bfloat16))
        in_maps.append({"xT": xTb, "Wsd": Wsd, "Wrep": Wrep, "maskT": maskTb})
    return in_maps


def kernel(x, A, W, a):
    from concourse.bass_utils import run_bass_kernel_spmd

    nc = _build()
    in_maps = _prep_inputs(x, A, W, a)
    res = run_bass_kernel_spmd(nc, in_maps, list(range(B)))
    out = np.stack([np.asarray(res.results[b]["y"]).astype(np.float32) for b in range(B)])
    return out


# revision 53
# speedup vs baseline: 1.4018x; 1.0038x over previous
"""GATv2-style masked attention kernel for Trainium2, 8-core data-parallel over batch.

Per core (one batch element, N=2048 nodes, F=256 features):
  h = x @ W                              (PE, fp16)
  s_src = h @ a[:F], s_dst = h @ a[F:]   (PE, fused into the same matmuls)
  e[i,j] = leaky_relu(s_src[i] + s_dst[j], 0.2), masked by A
  alpha = softmax_j(e); y = alpha @ h

Softmax without row maxima: any per-i factor cancels in the normalization
y = (P @ [h|1]) -> y[:, :F] / y[:, F].  Since exp is monotone,
  P[j,i] = exp(max(-0.8*s_src_i, 0.8*s_dst_j) + 0.2*s_dst_j - 54)
         = max(E_i, F_j) * G_j
with E_i = exp(-0.8*s_src_i - 27), F_j = exp(0.8*s_dst_j - 27),
G_j = exp(0.2*s_dst_j - 27): the N x N exp stream disappears entirely --
only per-node exp vectors are needed.  Per score chunk the DVE computes
(E max F_j) * G_j as one 4x-mode tensor_scalar, then the {0,1} mask is
applied with one 2x tensor_tensor; some mask multiplies run on the
otherwise-idle GPSIMD engine.  The -54 recentering (3.4*sigma with
sigma = ||W @ a_dst|| ~= 16 for this randn input spec) keeps the bf16
score tiles centered where the big softmax weights live.

Scores are built transposed ([j, i]) so the P @ h contraction has j on
partitions.  The i range is processed in two waves of 8 PSUM banks each.
h is produced two i-chunks per PSUM bank ([W] only); s_dst accumulates in a
16-column bank via rank-1 matmuls against the w_dst column so the F/G exps
batch into four activations; h->SBUF drains are spread over ACT and DVE
(GPSIMD cannot touch PSUM) so the banks recycle quickly into the wave
accumulators.  The last few j-strips of each wave run bank-major so
normalization and the bf16 output stores pipeline behind the matmul tail
instead of serializing after it.  A stream of tiny junk matmuls at t=0
rides out the PE p-state ramp so the real matmul stream runs at full
clock.  The host supplies: x transposed in fp16, the mask transposed as
bf16 {0,1}, [W | W@a_dst | 0] in fp16, and W@a_src replicated across 128
columns (pure layout/precision transforms); y is stored bf16 and upcast
on the host.
"""

import numpy as np

B, N, F = 8, 2048, 256
PC = N // 128        # 16 j-chunks
CC = 2               # contraction chunks (f in two halves of 128)
HALF = N // 2
NWARM = 52
STAG = 4             # last STAG j-strips of each wave run bank-major
_CACHE = {}


def _build():
    if "nc" in _CACHE:
        return _CACHE["nc"]

    from contextlib import ExitStack
    import concourse.bacc as bacc
    import concourse.tile as tile
    import concourse.mybir as mybir

    dt = mybir.dt
    AF = mybir.ActivationFunctionType
    ALU = mybir.AluOpType

    nc = bacc.Bacc("TRN2", target_bir_lowering=False, debug=False, num_devices=B)

    xT = nc.dram_tensor("xT", [F, N], dt.float16, kind="ExternalInput").ap()
    Wsd = nc.dram_tensor("Wsd", [F, F + 2], dt.float16, kind="ExternalInput").ap()
    Wrep = nc.dram_tensor("Wrep", [F, 128], dt.float16, kind="ExternalInput").ap()
    maskT = nc.dram_tensor("maskT", [N, N], dt.bfloat16, kind="ExternalInput").ap()
    y = nc.dram_tensor("y", [N, F], dt.bfloat16, kind="ExternalOutput").ap()

    with tile.TileContext(nc) as tc, ExitStack() as ctx:
        sb = ctx.enter_context(tc.tile_pool(name="sb", bufs=1))
        tpool = ctx.enter_context(tc.tile_pool(name="tp", bufs=6))
        phpool = ctx.enter_context(tc.tile_pool(name="ph", bufs=8))
        ypool = ctx.enter_context(tc.tile_pool(name="ysb", bufs=8))
        spool = ctx.enter_context(tc.tile_pool(name="small", bufs=4))
        ps = ctx.enter_context(tc.tile_pool(name="ps", bufs=8, space="PSUM"))

        # ---- persistent SBUF tensors ----
        xh = [
            sb.tile([128, CC, 1024], dt.float16, tag=f"xh{i}", name=f"xh{i}")
            for i in range(2)
        ]  # x in two halves of 8 i-chunks each
        Wsd_t = sb.tile([128, CC, F + 2], dt.float16, tag="Wsd")
        Wrep_t = sb.tile([128, CC, 128], dt.float16, tag="Wrep")
        maskS = [
            sb.tile([128, HALF], dt.bfloat16, tag=f"maskS{j}", name=f"maskS{j}")
            for j in range(2 * PC)
        ]
        hh = sb.tile([128, PC, F + 2], dt.float16, tag="hh")    # [h | 1] per chunk
        E2 = [
            sb.tile([128, HALF], dt.bfloat16, tag=f"E2_{i}", name=f"E2_{i}")
            for i in range(2)
        ]  # exp(-0.8*s_src - 27) replicated, per i-half
        Fv = sb.tile([128, PC], dt.float32, tag="Fv")  # exp(0.8*s_dst - 27)
        Gv = sb.tile([128, PC], dt.float32, tag="Gv")  # exp(0.2*s_dst - 27)
        cst = sb.tile([128, 4], dt.float32, tag="cst")  # -27, -0.8, 0.8, 0.2
        junk = sb.tile([128, 64], dt.float16, tag="junk")

        nc.gpsimd.memset(junk[:], 0.0)
        nc.gpsimd.memset(cst[:, 0:1], -27.0)
        nc.gpsimd.memset(cst[:, 1:2], -0.8)
        nc.gpsimd.memset(cst[:, 2:3], 0.8)
        nc.gpsimd.memset(cst[:, 3:4], 0.2)
        nc.vector.memset(hh[:, :, F : F + 1], 1.0)
        # Pull the Exp activation table in before the DMA window closes.
        scr = spool.tile([128, 1], dt.float32, tag="rec", name="scr")
        nc.scalar.activation(scr[:], cst[:, 0:1], AF.Exp, bias=0.0, scale=1.0)

        # ---- PE p-state warm-up ----
        warm = ps.tile([64, 64], dt.float32, tag="bank", name="warm")
        for _ in range(NWARM):
            nc.tensor.matmul(warm[:], junk[:, 0:64], junk[:, 0:64], start=True, stop=True)

        # ---- DMAs ----
        xTr = xT.rearrange("(c p) n -> p c n", p=128)

        def load_mask(w, j):
            nc.sync.dma_start(
                maskS[w * PC + j][:],
                maskT[j * 128 : (j + 1) * 128, w * HALF : (w + 1) * HALF],
            )

        def load_xq(q):
            h, off = divmod(q * 512, 1024)
            nc.sync.dma_start(
                xh[h][:, :, off : off + 512], xTr[:, :, q * 512 : q * 512 + 512]
            )

        load_xq(0)
        nc.sync.dma_start(Wsd_t[:], Wsd.rearrange("(c p) m -> p c m", p=128))
        nc.sync.dma_start(Wrep_t[:], Wrep.rearrange("(c p) m -> p c m", p=128))
        load_xq(1)
        load_xq(2)
        load_mask(0, 0)
        load_xq(3)
        load_mask(0, 1)
        for j in range(2, PC):
            load_mask(0, j)
        for j in range(PC):
            load_mask(1, j)

        # ---- preamble: E / h / s_dst production ----
        def emit_seg_mm(seg):
            rp = ps.tile([128, 512], dt.float32, tag="bank", name=f"rep{seg}")
            for c in range(CC):
                nc.tensor.matmul(
                    rp[:],
                    Wrep_t[:, c, :],
                    xh[seg // 2][:, c, (seg % 2) * 512 : (seg % 2) * 512 + 512],
                    start=(c == 0),
                    stop=(c == CC - 1),
                )
            return rp

        def emit_seg_drain(seg, rp):
            half, off = divmod(seg * 512, HALF)
            nc.scalar.activation(
                E2[half][:, off : off + 512], rp[:], AF.Exp,
                bias=cst[:, 0:1], scale=cst[:, 1:2],
            )

        def emit_H_mm(nb):
            # paired h for chunks 2nb, 2nb+1 -> one [128, 512] bank
            hb = ps.tile([128, 512], dt.float32, tag="bank", name=f"H{nb}")
            for half in range(2):
                ch = 2 * nb + half
                q, off = divmod(ch * 128, 1024)
                for c in range(CC):
                    nc.tensor.matmul(
                        hb[:, half * 256 : half * 256 + 256],
                        xh[q][:, c, off : off + 128],
                        Wsd_t[:, c, 0:256],
                        start=(c == 0),
                        stop=(c == CC - 1),
                    )
            return hb

        def emit_sdb_mms(sdb, ch):
            q, off = divmod(ch * 128, 1024)
            for c in range(CC):
                nc.tensor.matmul(
                    sdb[:, ch : ch + 1],
                    xh[q][:, c, off : off + 128],
                    Wsd_t[:, c, F : F + 1],
                    start=(c == 0),
                    stop=(c == CC - 1),
                )

        def emit_hh_drain(nb, hb, eng):
            # GPSIMD cannot read PSUM -- ACT/DVE only here.
            dst = hh[:, 2 * nb : 2 * nb + 2, 0:F]
            if eng == "act":
                nc.scalar.copy(dst, hb[:])
            else:
                nc.vector.tensor_copy(dst, hb[:])

        def emit_fg(sdb, lo, hi):
            nc.scalar.activation(
                Fv[:, lo:hi], sdb[:, lo:hi], AF.Exp, bias=cst[:, 0:1], scale=cst[:, 2:3],
            )
            nc.scalar.activation(
                Gv[:, lo:hi], sdb[:, lo:hi], AF.Exp, bias=cst[:, 0:1], scale=cst[:, 3:4],
            )

        # x quarters 0,1: chunks 0-7, segs 0-1; sdb is long-lived
        H = {}
        H[0] = emit_H_mm(0)
        H[1] = emit_H_mm(1)
        rp0 = emit_seg_mm(0)
        emit_seg_drain(0, rp0)
        sdb = ps.tile([128, PC], dt.float32, tag="bank", name="sdb")
        for ch in range(0, 4):
            emit_sdb_mms(sdb, ch)
        emit_hh_drain(0, H[0], "dve")
        rp1 = emit_seg_mm(1)
        emit_seg_drain(1, rp1)
        H[2] = emit_H_mm(2)
        H[3] = emit_H_mm(3)
        for ch in range(4, 8):
            emit_sdb_mms(sdb, ch)
        emit_fg(sdb, 0, 8)
        emit_hh_drain(1, H[1], "dve")
        emit_hh_drain(2, H[2], "dve")

        # x quarter 2: seg 2, chunks 8-11
        rp2 = emit_seg_mm(2)
        emit_seg_drain(2, rp2)
        H[4] = emit_H_mm(4)
        H[5] = emit_H_mm(5)
        for ch in range(8, 12):
            emit_sdb_mms(sdb, ch)
        emit_hh_drain(3, H[3], "dve")

        # x quarter 3: seg 3, s_dst 12-15, chunks 12-15
        rp3 = emit_seg_mm(3)
        for ch in range(12, 16):
            emit_sdb_mms(sdb, ch)
        emit_fg(sdb, 8, 16)
        emit_seg_drain(3, rp3)
        H[6] = emit_H_mm(6)
        H[7] = emit_H_mm(7)
        emit_hh_drain(4, H[4], "act")
        emit_hh_drain(6, H[6], "act")
        emit_hh_drain(5, H[5], "act")
        # hh drain for bank 7 rides inside wave 1 on the DVE (below)

        # ---- score strips ----
        POOL_KS = ((2, 5, 8, 11), (4, 8, 12))

        def make_ts(w, k):
            t = tpool.tile([128, HALF], dt.bfloat16, tag="t", name=f"t{w}_{k}")
            nc.vector.tensor_scalar(
                t[:], E2[w][:], Fv[:, k : k + 1], Gv[:, k : k + 1],
                op0=ALU.max, op1=ALU.mult,
            )
            return t

        def make_ph(w, k, t):
            ph = phpool.tile([128, HALF], dt.bfloat16, tag="ph", name=f"ph{w}_{k}")
            if k in POOL_KS[w]:
                # halves: the strip's first 4 bank-matmuls only need cols 0:512
                nc.gpsimd.tensor_tensor(ph[:, 0:512], t[:, 0:512], maskS[w * PC + k][:, 0:512], op=ALU.mult)
                nc.gpsimd.tensor_tensor(ph[:, 512:1024], t[:, 512:1024], maskS[w * PC + k][:, 512:1024], op=ALU.mult)
            else:
                nc.vector.tensor_mul(ph[:], t[:], maskS[w * PC + k][:])
            return ph

        def emit_mms(banks, ph, k, order=None):
            for ic in (order if order is not None else range(8)):
                nc.tensor.matmul(
                    banks[ic][:, 0 : F + 1],
                    ph[:, ic * 128 : (ic + 1) * 128],
                    hh[:, k, 0 : F + 1],
                    start=(k == 0),
                    stop=(k == PC - 1),
                )

        def emit_norm(ysb, sl, bank, eng):
            rec = spool.tile([128, 1], dt.float32, tag="rec")
            nc.vector.reciprocal(rec[:], bank[:, F : F + 1])
            if eng == "act":
                nc.scalar.activation(ysb[:, sl, :], bank[:, 0:F], AF.Copy, bias=0.0, scale=rec[:, 0:1])
            else:
                nc.vector.tensor_scalar_mul(ysb[:, sl, :], bank[:, 0:F], rec[:, 0:1])

        def emit_wave(w, ybanks, k0_order=None, dve_hooks=None):
            """j-strips 0..PC-1; last STAG strips bank-major with fused norms+stores."""
            t_next = make_ts(w, 0)
            for k in range(PC - STAG):
                t = t_next
                t_next = make_ts(w, k + 1)
                ph = make_ph(w, k, t)
                if dve_hooks and k in dve_hooks:
                    dve_hooks[k]()
                emit_mms(ybanks, ph, k, order=(k0_order if k == 0 else None))
            phs = {}
            for k in range(PC - STAG, PC):
                t = t_next
                if k + 1 < PC:
                    t_next = make_ts(w, k + 1)
                phs[k] = make_ph(w, k, t)
            groups = [(0, 2), (2, 2), (4, 2), (6, 2)] if w == 0 else [(0, 4), (4, 3), (7, 1)]
            ysbs = {g[0]: ypool.tile([128, g[1], F], dt.bfloat16, tag="ysb", name=f"ysb{w}_{g[0]}")
                    for g in groups}
            gof = {}
            for g0, gn in groups:
                for i in range(g0, g0 + gn):
                    gof[i] = (g0, gn)
            # wave 0's scale-copies all ride ACT so the DVE stream rolls
            # straight into wave 1's strips; the final wave alternates.
            engs = ("act",) if w == 0 else ("dve", "act")
            for ic in range(8):
                for k in range(PC - STAG, PC):
                    nc.tensor.matmul(
                        ybanks[ic][:, 0 : F + 1],
                        phs[k][:, ic * 128 : (ic + 1) * 128],
                        hh[:, k, 0 : F + 1],
                        start=False,
                        stop=(k == PC - 1),
                    )
                g0, gn = gof[ic]
                emit_norm(ysbs[g0], ic - g0, ybanks[ic], engs[ic % len(engs)])
                if ic == g0 + gn - 1:
                    lo = w * HALF + g0 * 128
                    nc.sync.dma_start(
                        y[lo : lo + gn * 128, :].rearrange("(c p) f -> p c f", p=128),
                        ysbs[g0][:],
                    )

        # wave 1 -- k0 banks ordered by PSUM-slot drain readiness; hh drains for
        # H5/H7 ride the DVE between early strips
        ybanks1 = [
            ps.tile([128, F + 2], dt.float32, tag="bank", name=f"yb0_{i}")
            for i in range(8)
        ]
        # ps slot children (alloc order warm,H0,H1,rep0,sdb,rep1,H2,H3,rep2,H4,H5,rep3,H6,H7):
        # yb0_0<-H2, yb0_1<-H3, yb0_2<-rep2, yb0_3<-H4, yb0_4<-H5, yb0_5<-rep3, yb0_6<-H6, yb0_7<-H7
        hooks = {
            3: lambda: emit_hh_drain(7, H[7], "dve"),
        }
        emit_wave(0, ybanks1, k0_order=[0, 2, 1, 3, 5, 6, 4, 7], dve_hooks=hooks)
        ybanks2 = [
            ps.tile([128, F + 2], dt.float32, tag="bank", name=f"yb1_{i}")
            for i in range(8)
        ]
        emit_wave(1, ybanks2)

    nc.compile()
    _CACHE["nc"] = nc
    return nc


def _prep_inputs(x, A, W, a):
    """Host-side layout/precision transforms (per batch element)."""
    import ml_dtypes

    W32 = np.asarray(W, dtype=np.float32)
    a32 = np.asarray(a, dtype=np.float32)
    w_src = W32 @ a32[:F]
    w_dst = W32 @ a32[F:]
    Wsd = np.ascontiguousarray(
        np.concatenate([W32, w_dst[:, None], np.zeros((F, 1), np.float32)], axis=1)
    ).astype(np.float16)
    Wrep = np.ascontiguousarray(np.tile(w_src[:, None], (1, 128))).astype(np.float16)
    in_maps = []
    for b in range(B):
        xTb = np.ascontiguousarray(np.asarray(x[b], dtype=np.float32).T).astype(np.float16)
        maskTb = np.ascontiguousarray((np.asarray(A[b]).T > 0).astype(ml_dtypes.bfloat16))
        in_maps.append({"xT": xTb, "Wsd": Wsd, "Wrep": Wrep, "maskT": maskTb})
    return in_maps


def kernel(x, A, W, a):
    from concourse.bass_utils import run_bass_kernel_spmd

    nc = _build()
    in_maps = _prep_inputs(x, A, W, a)
    res = run_bass_kernel_spmd(nc, in_maps, list(range(B)))
    out = np.stack([np.asarray(res.results[b]["y"]).astype(np.float32) for b in range(B)])
    return out
